# revision 40
# baseline (speedup 1.0000x reference)
"""Trainium2 Bass kernel for the ChebConv GNN problem
(nn_ChebConvConvolutional): 2x GCNConv + 1x ChebConv(K=3), N=10000 nodes,
E=160000 edges, F=512, celu activations.

Strategy (8 NeuronCores, SPMD):
  * Nodes are sharded 1250/core (padded to 1280). Edges are sharded by
    destination core and grouped into 128-dest tiles; per dest-tile the
    source nodes are deduplicated and the edge weights are baked into dense
    [128 src x 128 dst] one-hot "S" matrices (GCN self-loops folded in as
    edges with value dinv^2, Cheb normalization negated so the scatter
    directly produces lhat).
  * Every graph op is computed aggregate-first: h = celu((A @ x) @ W + b),
    so layer 1 needs no collective (x replicated); layers end with a small
    AllGather of the core's 1280x512 bf16 slice.
  * On device, per dest-tile: dma_gather pulls the (bf16) feature rows of
    the deduped sources; the tensor engine computes
    psumT[f, d] += msgs_chunk[e, f].T @ S[e, d] (feature-major aggregate),
    then the dense GEMM out[n, fo] += aggT_k.T @ W_k (node-major), and
    celu = max(z,0) + min(exp(z)-1, 0) runs on ACT + DVE.
  * ChebConv K=3 is restructured GEMM-first using linearity of lhat
    (lhat(z) @ W == lhat(z @ W)):
      Y1 = h2 @ Wk1, Y2 = h2 @ (2*Wk2)          (local GEMMs, width 256 each)
      [A1|A2] = lhat([Y1|Y2])                   (one 512-wide scatter pass)
      A3 = lhat(A2)                             (one 256-wide scatter pass)
      out = celu(h2 @ (Wk0-Wk2) + A1 + A3 + bc)
    This removes the h2 allgather (replaced by the same-size Y12 one),
    halves the second cheb pass width (gather bytes + PE cycles), and
    drops the per-pass feature-major transposes.
"""
import numpy as np
import ml_dtypes

import concourse.bacc as bacc
import concourse.mybir as mybir
import concourse.tile as tile
from concourse import library_config
from concourse.bass_utils import run_bass_kernel_spmd
from concourse.tile import add_dep_helper

BF16 = ml_dtypes.bfloat16
FP32 = mybir.dt.float32
BF16D = mybir.dt.bfloat16
I16 = mybir.dt.int16

P = 8            # cores
N = 10000        # nodes
NPC = N // P     # nodes per core
NPAD = 1280      # padded nodes per core
NTOT = NPAD * P
F = 512          # feature width of x / h1 / h2
DOUT = 256
DT = 128         # dests per dest tile
NDT = NPAD // DT # dest tiles per core
KC = F // 128    # contraction chunks (4)
NCH = 1          # AllGather chunks per layer (chunking serializes; 1 is best)
CH = NPAD // NCH # local rows per AG chunk (256)


# ----------------------------------------------------------------- host prep

def _to_padded_id(n):
    """Global node id -> row in the chunked-AllGather global layout:
    [NCH chunks][P ranks][CH rows]."""
    r = n // NPC
    l = n % NPC
    j = l // CH
    return j * (P * CH) + r * CH + (l % CH)


def _build_edge_tiles(src, dst, val):
    """Shard by dest core, tile by 128 dests, dedup sources per tile.
    Returns (ET [NDT], idx [P, T, 128] int32 padded ids, S [P, T, 128, DT])."""
    per_core = []
    order = np.argsort(dst, kind="stable")
    src, dst, val = src[order], dst[order], val[order]
    core_of = dst // NPC
    core_starts = np.searchsorted(core_of, np.arange(P + 1))
    for c in range(P):
        lo, hi = core_starts[c], core_starts[c + 1]
        s, d, v = src[lo:hi], dst[lo:hi] - c * NPC, val[lo:hi]
        tile_of = d // DT
        tile_starts = np.searchsorted(tile_of, np.arange(NDT + 1))
        groups = []
        for t in range(NDT):
            a, b = tile_starts[t], tile_starts[t + 1]
            st, dl, vt = s[a:b], d[a:b] - t * DT, v[a:b]
            uniq, inv = np.unique(st, return_inverse=True)
            if len(uniq) == 0:
                groups.append((np.zeros(1, np.int64), np.zeros((1, DT), np.float32)))
                continue
            S = np.zeros((len(uniq), DT), np.float32)
            np.add.at(S, (inv, dl), vt)
            groups.append((uniq, S))
        per_core.append(groups)

    ET = [max(max((len(per_core[c][t][0]) + 127) // 128, 1) for c in range(P))
          for t in range(NDT)]
    T = sum(ET)
    off = np.cumsum([0] + ET[:-1])
    idx = np.zeros((P, T, 128), np.int32)
    S_all = np.zeros((P, T, 128, DT), np.float32)
    for c in range(P):
        for t in range(NDT):
            uniq, S = per_core[c][t]
            n = len(uniq)
            o = off[t]
            idx[c, o:o + (n + 127) // 128].reshape(-1)[:n] = _to_padded_id(uniq)
            S_all[c, o:o + (n + 127) // 128].reshape(-1, DT)[:n] = S
    return tuple(ET), idx, S_all


def _idx_dev(idx_core):
    """[T, 128] int32 -> [128, T*8] int16 (wrap 16 partitions, replicate x8)."""
    flat = idx_core.reshape(-1)
    n = len(flat)
    a = np.zeros((16, n // 16), np.int16)
    a[np.arange(n) % 16, np.arange(n) // 16] = flat.astype(np.int16)
    return np.tile(a, (8, 1))


def _s_dev(S_core):
    """[T, 128, DT] -> [128, T*DT] bf16."""
    T = S_core.shape[0]
    return np.ascontiguousarray(
        S_core.transpose(1, 0, 2).reshape(128, T * DT)).astype(BF16)


def _w_dev(W):
    """[F, fo] -> [128, KC*fo] bf16 (chunk k at cols [k*fo, (k+1)*fo))."""
    fi, fo = W.shape
    k = fi // 128
    return np.ascontiguousarray(
        W.reshape(k, 128, fo).transpose(1, 0, 2).reshape(128, k * fo)).astype(BF16)


def _prep(x, edge_index, edge_weight, W1, b1, W2, b2, Wc, bc):
    row = np.asarray(edge_index[0], np.int64)
    col = np.asarray(edge_index[1], np.int64)
    w = np.asarray(edge_weight, np.float32)

    # GCN norm (layers 1 & 2): deg over dest (col) + 1 self loop.
    deg = np.zeros(N, np.float32)
    np.add.at(deg, col, w)
    deg += 1.0
    dinv = (1.0 / np.sqrt(deg)).astype(np.float32)
    g_src = np.concatenate([row, np.arange(N)])
    g_dst = np.concatenate([col, np.arange(N)])
    g_val = np.concatenate([dinv[row] * w * dinv[col], dinv * dinv]).astype(np.float32)

    # Cheb: drop self loops, deg over src (row), negate (lhat = -A_norm).
    keep = row != col
    r0, c0, w0 = row[keep], col[keep], w[keep]
    deg2 = np.zeros(N, np.float32)
    np.add.at(deg2, r0, w0)
    dinv2 = np.where(deg2 > 0, 1.0 / np.sqrt(deg2), 0.0).astype(np.float32)
    c_val = -(dinv2[r0] * w0 * dinv2[c0]).astype(np.float32)

    ETg, idxg, Sg = _build_edge_tiles(g_src, g_dst, g_val)
    ETc, idxc, Sc = _build_edge_tiles(r0, c0, c_val)

    x = np.asarray(x, np.float32)
    x_pad = np.zeros((NTOT, F), BF16)
    x_pad[_to_padded_id(np.arange(N))] = x.astype(BF16)

    Wc = np.asarray(Wc, np.float32)
    com = dict(
        x_bf=x_pad,
        w1=_w_dev(np.asarray(W1, np.float32)),
        w2=_w_dev(np.asarray(W2, np.float32)),
        wa=_w_dev(Wc[0] - Wc[2]),
        wpq=_w_dev(np.concatenate([Wc[1], 2.0 * Wc[2]], axis=1)),
        ident=np.eye(128, dtype=BF16),
    )
    biases = (np.asarray(b1, np.float32), np.asarray(b2, np.float32),
              np.asarray(bc, np.float32))
    in_maps = []
    for c in range(P):
        m = dict(com)
        m["idxg"] = _idx_dev(idxg[c])
        m["sg"] = _s_dev(Sg[c])
        m["idxc"] = _idx_dev(idxc[c])
        m["sc"] = _s_dev(Sc[c])
        in_maps.append(m)
    return ETg, ETc, biases, in_maps


# ------------------------------------------------------------- bass program

_CACHE = {}


def _build_program(ETg, ETc, has_bias):
    import os
    key = (ETg, ETc, has_bias, os.environ.get("GNN_PHASES", "9"))
    if key in _CACHE:
        return _CACHE[key]
    TG, TC = sum(ETg), sum(ETc)
    ETMAX = max(max(ETg), max(ETc))

    nc = bacc.Bacc("TRN2", target_bir_lowering=False, num_devices=P,
                   num_swdge_queues=4)
    x_bf = nc.dram_tensor("x_bf", [NTOT, F], BF16D, kind="ExternalInput")
    idxg = nc.dram_tensor("idxg", [128, TG * 8], I16, kind="ExternalInput")
    sg = nc.dram_tensor("sg", [128, TG * DT], BF16D, kind="ExternalInput")
    idxc = nc.dram_tensor("idxc", [128, TC * 8], I16, kind="ExternalInput")
    sc = nc.dram_tensor("sc", [128, TC * DT], BF16D, kind="ExternalInput")
    w1 = nc.dram_tensor("w1", [128, KC * F], BF16D, kind="ExternalInput")
    w2 = nc.dram_tensor("w2", [128, KC * F], BF16D, kind="ExternalInput")
    wa = nc.dram_tensor("wa", [128, KC * DOUT], BF16D, kind="ExternalInput")
    wpq = nc.dram_tensor("wpq", [128, KC * F], BF16D, kind="ExternalInput")
    ident = nc.dram_tensor("ident", [128, 128], BF16D, kind="ExternalInput")
    if has_bias:
        brows = nc.dram_tensor("brows", [1, 2 * F + DOUT], FP32, kind="ExternalInput")
    outp = nc.dram_tensor("out", [NPAD, DOUT], FP32, kind="ExternalOutput")

    h1c = nc.dram_tensor("h1c", [NPAD, F], BF16D, kind="Internal")
    h1f = nc.dram_tensor("h1f", [NTOT, F], BF16D, kind="Internal", addr_space="Shared")
    y12c = nc.dram_tensor("y12c", [NPAD, F], BF16D, kind="Internal")
    y12f = nc.dram_tensor("y12f", [NTOT, F], BF16D, kind="Internal", addr_space="Shared")
    a2c = nc.dram_tensor("a2c", [NPAD, DOUT], BF16D, kind="Internal")
    a2f = nc.dram_tensor("a2f", [NTOT, DOUT], BF16D, kind="Internal", addr_space="Shared")

    Exp = mybir.ActivationFunctionType.Exp
    Alu = mybir.AluOpType

    with tile.TileContext(nc) as tc:
        with (
            tc.tile_pool(name="const", bufs=1) as cpool,
            tc.tile_pool(name="keep", bufs=1) as kpool,
            tc.tile_pool(name="msgs", bufs=2) as mpool,
            tc.tile_pool(name="msgsn", bufs=3) as npool,
            tc.tile_pool(name="work", bufs=3) as wpool,
            tc.tile_pool(name="psum", bufs=2, space="PSUM") as ppool,
            tc.tile_pool(name="psum3", bufs=4, space="PSUM") as ppool3,
        ):
            lib = nc.gpsimd.load_library(library_config.mlp)
            prep_sems = [nc.alloc_semaphore(f"prep_dma{q}") for q in range(4)]
            sem_clrs = [nc.gpsimd.sem_clear(s) for s in prep_sems]
            first_prep = [0]

            # Load order is priority order on the HWDGE rings: everything
            # layer-1 tile 0 needs first, then the rest.
            ig_sb = cpool.tile([128, TG * 8], I16, tag="ig")
            nc.sync.dma_start(ig_sb[:], idxg[:])
            id_sb = cpool.tile([128, 128], BF16D, tag="id")
            nc.sync.dma_start(id_sb[:], ident[:])

            sg_sb = cpool.tile([128, TG * DT], BF16D, tag="sg")
            sc_sb = cpool.tile([128, TC * DT], BF16D, tag="sc")
            offg = np.cumsum([0] + list(ETg[:-1]))
            offc = np.cumsum([0] + list(ETc[:-1]))
            nc.sync.dma_start(sg_sb[:, :ETg[0] * DT], sg[:, :ETg[0] * DT])
            w1_sb = cpool.tile([128, KC * F], BF16D, tag="w1")
            nc.sync.dma_start(w1_sb[:], w1[:])
            ic_sb = cpool.tile([128, TC * 8], I16, tag="ic")
            nc.sync.dma_start(ic_sb[:], idxc[:])
            for t in range(NDT):
                a, b = offg[t] * DT, (offg[t] + ETg[t]) * DT
                if t > 0:
                    nc.sync.dma_start(sg_sb[:, a:b], sg[:, a:b])
                a, b = offc[t] * DT, (offc[t] + ETc[t]) * DT
                nc.sync.dma_start(sc_sb[:, a:b], sc[:, a:b])
            w2_sb = cpool.tile([128, KC * F], BF16D, tag="w2")
            nc.sync.dma_start(w2_sb[:], w2[:])
            wa_sb = cpool.tile([128, KC * DOUT], BF16D, tag="wa")
            nc.sync.dma_start(wa_sb[:], wa[:])
            wpq_sb = cpool.tile([128, KC * F], BF16D, tag="wpq")
            nc.sync.dma_start(wpq_sb[:], wpq[:])
            if has_bias:
                br_sb = cpool.tile([1, 2 * F + DOUT], FP32, tag="br")
                nc.sync.dma_start(br_sb[:], brows[:])
                ones_sb = cpool.tile([1, 128], FP32, tag="ones")
                nc.vector.memset(ones_sb[:], 1.0)

            h2keep = kpool.tile([128, NDT, F], BF16D, tag="h2k")
            a12keep = kpool.tile([128, NDT, F], BF16D, tag="a12k")

            first_gather = [0]
            qctr = [0]

            def gather_msgs(src_dram, ET, off, idx_sb, t, width, prep=False):
                """Gather the deduped source rows for dest-tile t (width cols).
                Split across SWDGE queues so descriptor generation runs on
                multiple Q7 core pairs in parallel. With prep=True the calls
                are prepare_only: descriptor generation runs as soon as the
                msgs slot frees (Tile defers the src_dram read dep to the
                trigger), so descgen overlaps the preceding allgather."""
                o = off[t]
                et = ET[t]
                if width == F:
                    msgs = mpool.tile([128, ETMAX, width], BF16D, tag="msgs")
                else:
                    msgs = npool.tile([128, ETMAX, width], BF16D, tag="msgsn")
                nq = min(8, et)
                bounds = [et * i // nq for i in range(nq + 1)]
                used = []
                for a, b in zip(bounds[:-1], bounds[1:]):
                    if b <= a:
                        continue
                    q = qctr[0] % 4
                    qctr[0] += 1
                    used.append(q)
                    if prep:
                        prep_pending[q] += 1
                    gi = nc.gpsimd.dma_gather(
                        msgs[:, a:b, :], src_dram[:],
                        idx_sb[:, (o + a) * 8:(o + b) * 8],
                        (b - a) * 128, (b - a) * 128, width,
                        single_packet=True, queue_num=q, prepare_only=prep,
                        sem=prep_sems[q] if prep else None)
                    if prep and first_prep[0] < 8:
                        add_dep_helper(gi.ins, sem_clrs[q].ins,
                                       reason="clear prep sem before preps")
                        first_prep[0] += 1
                    if first_gather[0] < 4:
                        add_dep_helper(gi.ins, lib.ins,
                                       reason="mlp lib before gather")
                        first_gather[0] += 1
                return msgs, sorted(set(used))

            prep_pending = [0, 0, 0, 0]
            prep_fired = [0, 0, 0, 0]

            def fire(queues):
                for q in sorted(set(queues)):
                    nc.gpsimd.trigger_dma(count=None, queue_num=q)
                    prep_fired[q] += prep_pending[q]
                    prep_pending[q] = 0

            def prefetch(src_dram, ET, off, idx_sb, width, n):
                """prepare_only-gather the first n tiles of a pass: descgen
                happens inside the allgather window, the DMAs launch the
                moment the allgather lands. Consumer sync is user-managed
                for preps: returns (msgs, [(q, sem_val)]) guard pairs the
                consuming matmuls must wait_ge on."""
                pre = []
                for t in range(n):
                    m, u = gather_msgs(src_dram, ET, off, idx_sb, t, width,
                                       prep=True)
                    fire(u)
                    guards = [(q, 16 * prep_fired[q]) for q in u]
                    pre.append((m, guards))
                return pre

            def scatter_mms(ps, msgs, ET, off, s_sb, t, start, stop,
                            guards=()):
                # S tile as stationary lhsT, msgs streamed as rhs (N=width):
                # ps[d, f] += S[e, d].T @ msgs[e, f]  (node-major aggregate).
                # guards: prep-DMA (q, sem_val) pairs the first matmul must
                # wait on (prep data-landed sync is user-managed).
                o = off[t]
                et = ET[t]
                waits = [nc.tensor.wait_ge(prep_sems[q], v)
                         for q, v in guards]
                for g in range(et):
                    mm = nc.tensor.matmul(
                        ps[:],
                        s_sb[:, (o + g) * DT:(o + g + 1) * DT],
                        msgs[:, g, :],
                        start=(start and g == 0), stop=(stop and g == et - 1))
                    if g == 0:
                        for w in waits:
                            add_dep_helper(mm.ins, w.ins,
                                           reason="prep gather landed")

            def scatter(src_dram, ET, off, idx_sb, s_sb, t, width=F,
                        pre=None):
                guards = ()
                if pre is not None and t < len(pre):
                    msgs, guards = pre[t]
                else:
                    msgs, _ = gather_msgs(src_dram, ET, off, idx_sb, t, width)
                ps = ppool3.tile([128, width], FP32, tag="psT")
                scatter_mms(ps, msgs, ET, off, s_sb, t, True, True,
                            guards=guards)
                return ps

            def celu(z_ps, width, out_ap):
                """out = max(z,0) + min(exp(z)-1, 0); z read from PSUM.
                fp32 temporaries: bf16 here doubles the output error
                (exp(z)-1 cancellation) for no measured speed gain."""
                e = wpool.tile([128, F], FP32, tag="e")
                nc.scalar.activation(e[:, :width], z_ps, Exp)
                em = wpool.tile([128, F], FP32, tag="em")
                nc.vector.tensor_scalar(
                    em[:, :width], e[:, :width], 1.0, 0.0,
                    Alu.subtract, Alu.min)
                nc.vector.scalar_tensor_tensor(
                    out_ap, z_ps, 0.0, em[:, :width], Alu.max, Alu.add)

            def gemm_bias(z_ps, width, b_off):
                if has_bias:
                    nc.tensor.matmul(
                        z_ps, ones_sb[:],
                        br_sb[:, b_off:b_off + width],
                        start=False, stop=False)

            def allgather_chunk(cin, cout, j):
                nc.gpsimd.collective_compute(
                    "AllGather", Alu.bypass,
                    replica_groups=[list(range(P))],
                    ins=[cin[j * CH:(j + 1) * CH, :]],
                    outs=[cout[j * P * CH:(j + 1) * P * CH, :]])

            def gcn_layer(src_dram, w_sb, dst_dram, keep_tile, b_off, post,
                          pre=None):
                for t in range(NDT):
                    ps = scatter(src_dram, ETg, offg, ig_sb, sg_sb, t, pre=pre)
                    agg = wpool.tile([128, F], BF16D, tag="agg")
                    nc.vector.tensor_copy(agg[:], ps[:])
                    tps = ppool.tile([128, KC, 128], BF16D, tag="tps")
                    for k in range(KC):
                        nc.tensor.transpose(
                            tps[:, k, :], agg[:, k * 128:(k + 1) * 128], id_sb[:])
                    aggT = wpool.tile([128, KC, 128], BF16D, tag="aggT")
                    nc.vector.tensor_copy(aggT[:], tps[:])
                    z = ppool.tile([128, F], FP32, tag="z")
                    for k in range(KC):
                        nc.tensor.matmul(
                            z[:], aggT[:, k, :], w_sb[:, k * F:(k + 1) * F],
                            start=(k == 0), stop=(k == KC - 1))
                    gemm_bias(z[:], F, b_off)
                    if keep_tile is None:
                        h = wpool.tile([128, F], BF16D, tag="h")
                        celu(z[:], F, h[:])
                        nc.sync.dma_start(dst_dram[t * 128:(t + 1) * 128, :], h[:])
                    else:
                        celu(z[:], F, keep_tile[:, t, :])
                        if dst_dram is not None:
                            nc.sync.dma_start(dst_dram[t * 128:(t + 1) * 128, :],
                                              keep_tile[:, t, :])
                    if post is not None:
                        post(t)

            import os
            PH = int(os.environ.get("GNN_PHASES", "9"))

            def post1(t):
                if PH >= 2 and (t * 128 + 128) % CH == 0:
                    allgather_chunk(h1c, h1f, (t * 128 + 128) // CH - 1)

            def post2(t):
                # Transpose h2 tile in place to feature-major, then the
                # cheb pre-GEMM: y12 = h2 @ [Wk1 | 2*Wk2]  (node-major out).
                tps = ppool.tile([128, KC, 128], BF16D, tag="tps")
                for k in range(KC):
                    nc.tensor.transpose(
                        tps[:, k, :], h2keep[:, t, k * 128:(k + 1) * 128],
                        id_sb[:])
                nc.vector.tensor_copy(h2keep[:, t, :], tps[:])
                zy = ppool.tile([128, F], FP32, tag="z")
                for k in range(KC):
                    nc.tensor.matmul(
                        zy[:], h2keep[:, t, k * 128:(k + 1) * 128],
                        wpq_sb[:, k * F:(k + 1) * F],
                        start=(k == 0), stop=(k == KC - 1))
                ynm = wpool.tile([128, F], BF16D, tag="h")
                nc.vector.tensor_copy(ynm[:], zy[:])
                nc.sync.dma_start(y12c[t * 128:(t + 1) * 128, :], ynm[:])
                if PH >= 4 and (t * 128 + 128) % CH == 0:
                    allgather_chunk(y12c, y12f, (t * 128 + 128) // CH - 1)

            # ---- layer 1: h1 = celu((Ag @ x) @ W1 + b1)
            # (prepare_only descgen prefetch measured net-negative: the
            # trigger waits block the SWDGE queue FIFOs. PRE_N=0 disables.)
            PRE_N = 0
            pre1 = prefetch(x_bf, ETg, offg, ig_sb, F, PRE_N) or None
            gcn_layer(x_bf, w1_sb, h1c, None, 0, post1, pre=pre1)

            # ---- layer 2: h2 = celu((Ag @ h1) @ W2 + b2); keep h2 on chip
            # (feature-major after post2) and emit y12 = h2 @ [Wk1 | 2*Wk2].
            if PH >= 3:
                pre2 = prefetch(h1f, ETg, offg, ig_sb, F, PRE_N) or None
                gcn_layer(h1f, w2_sb, None, h2keep, F, post2, pre=pre2)

            # ---- cheb pass B: [A1|A2] = lhat([Y1|Y2]); keep on chip,
            # allgather A2 (256 wide) for pass C.
            if PH >= 5:
                preb = prefetch(y12f, ETc, offc, ic_sb, F, PRE_N) or None
                for t in range(NDT):
                    ps = scatter(y12f, ETc, offc, ic_sb, sc_sb, t, pre=preb)
                    nc.vector.tensor_copy(a12keep[:, t, :], ps[:])
                    nc.sync.dma_start(a2c[t * 128:(t + 1) * 128, :],
                                      a12keep[:, t, DOUT:])
                    if PH >= 6 and (t * 128 + 128) % CH == 0:
                        allgather_chunk(a2c, a2f, (t * 128 + 128) // CH - 1)

            # ---- cheb pass C + output:
            # out = celu(h2 @ (Wk0-Wk2) + A1 + lhat(A2) + bc)
            # Precompute zpre[t] = h2 @ (Wk0-Wk2) + bc + A1 into a12keep:
            # depends only on h2keep/a12keep, so the PE fills the a2
            # allgather window with it and the main loop below shrinks.
            for t in range(NDT if PH >= 7 else 0):
                zp = ppool.tile([128, DOUT], FP32, tag="z")
                for k in range(KC):
                    # h2keep is feature-major (transposed in post2)
                    nc.tensor.matmul(
                        zp[:], h2keep[:, t, k * 128:(k + 1) * 128],
                        wa_sb[:, k * DOUT:(k + 1) * DOUT],
                        start=(k == 0), stop=False)
                gemm_bias(zp[:], DOUT, 2 * F)
                nc.tensor.matmul(
                    zp[:], id_sb[:], a12keep[:, t, :DOUT],
                    start=False, stop=True)
                nc.vector.tensor_copy(a12keep[:, t, :DOUT], zp[:])
            prec = prefetch(a2f, ETc, offc, ic_sb, DOUT, PRE_N) if PH >= 7 else []
            for t in range(NDT if PH >= 7 else 0):
                zo = ppool.tile([128, DOUT], FP32, tag="z")
                guards = ()
                if t < len(prec):
                    msgs, guards = prec[t]
                else:
                    msgs, _ = gather_msgs(a2f, ETc, offc, ic_sb, t, DOUT)
                scatter_mms(zo, msgs, ETc, offc, sc_sb, t, True, False,
                            guards=guards)
                # zo += zpre via identity-stationary matmul (closes the group)
                nc.tensor.matmul(
                    zo[:], id_sb[:], a12keep[:, t, :DOUT],
                    start=False, stop=True)
                of = wpool.tile([128, DOUT], FP32, tag="of")
                celu(zo[:], DOUT, of[:])
                nc.sync.dma_start(outp[t * 128:(t + 1) * 128, :], of[:])

    nc.compile()
    _CACHE[key] = nc
    return nc


# ------------------------------------------------------------------- driver

def _run(inputs, trace=False, tmpdir=None):
    ETg, ETc, biases, in_maps = _prep(**inputs)
    has_bias = any(np.any(b != 0) for b in biases)
    if has_bias:
        brow = np.concatenate(biases).astype(np.float32)[None, :]
        for m in in_maps:
            m["brows"] = brow
    nc = _build_program(ETg, ETc, has_bias)
    res = run_bass_kernel_spmd(nc, in_maps, core_ids=list(range(P)),
                               trace=trace, tmpdir=tmpdir)
    out = np.concatenate(
        [res.results[c]["out"][:NPC] for c in range(P)], axis=0)
    return out.astype(np.float32), res


def kernel(**inputs) -> np.ndarray:
    out, _ = _run(inputs)
    return out



# revision 42
# speedup vs baseline: 1.2346x; 1.2346x over previous
"""Trainium2 Bass kernel for the ChebConv GNN problem
(nn_ChebConvConvolutional): 2x GCNConv + 1x ChebConv(K=3), N=10000 nodes,
E=160000 edges, F=512, celu activations.

Strategy (8 NeuronCores, SPMD):
  * Nodes are sharded 1250/core (padded to 1280). Edges are sharded by
    destination core and grouped into 128-dest tiles; per dest-tile the
    source nodes are deduplicated and the edge weights are baked into dense
    [128 src x 128 dst] one-hot "S" matrices (GCN self-loops folded in as
    edges with value dinv^2, Cheb normalization negated so the scatter
    directly produces lhat).
  * Every graph op is computed aggregate-first: h = celu((A @ x) @ W + b),
    so layer 1 needs no collective (x replicated); layers end with a small
    AllGather of the core's 1280x512 bf16 slice.
  * On device, per dest-tile: dma_gather pulls the (bf16) feature rows of
    the deduped sources; the tensor engine computes
    psumT[f, d] += msgs_chunk[e, f].T @ S[e, d] (feature-major aggregate),
    then the dense GEMM out[n, fo] += aggT_k.T @ W_k (node-major), and
    celu = max(z,0) + min(exp(z)-1, 0) runs on ACT + DVE.
  * ChebConv K=3 is restructured GEMM-first using linearity of lhat
    (lhat(z) @ W == lhat(z @ W)):
      Y1 = h2 @ Wk1, Y2 = h2 @ (2*Wk2)          (local GEMMs, width 256 each)
      [A1|A2] = lhat([Y1|Y2])                   (one 512-wide scatter pass)
      A3 = lhat(A2)                             (one 256-wide scatter pass)
      out = celu(h2 @ (Wk0-Wk2) + A1 + A3 + bc)
    This removes the h2 allgather (replaced by the same-size Y12 one),
    halves the second cheb pass width (gather bytes + PE cycles), and
    drops the per-pass feature-major transposes.
"""
import numpy as np
import ml_dtypes

import concourse.bacc as bacc
import concourse.mybir as mybir
import concourse.tile as tile
from concourse import library_config
from concourse.bass_utils import run_bass_kernel_spmd
from concourse.tile import add_dep_helper

BF16 = ml_dtypes.bfloat16
FP32 = mybir.dt.float32
BF16D = mybir.dt.bfloat16
I16 = mybir.dt.int16

P = 8            # cores
N = 10000        # nodes
NPC = N // P     # nodes per core
NPAD = 1280      # padded nodes per core
NTOT = NPAD * P
F = 512          # feature width of x / h1 / h2
DOUT = 256
DT = 128         # dests per dest tile
NDT = NPAD // DT # dest tiles per core
KC = F // 128    # contraction chunks (4)
NCH = 1          # AllGather chunks per layer (chunking serializes; 1 is best)
CH = NPAD // NCH # local rows per AG chunk (256)


# ----------------------------------------------------------------- host prep

def _to_padded_id(n):
    """Global node id -> row in the chunked-AllGather global layout:
    [NCH chunks][P ranks][CH rows]."""
    r = n // NPC
    l = n % NPC
    j = l // CH
    return j * (P * CH) + r * CH + (l % CH)


def _build_edge_tiles(src, dst, val):
    """Shard by dest core, tile by 128 dests, dedup sources per tile.
    Returns (ET [NDT], idx [P, T, 128] int32 padded ids, S [P, T, 128, DT])."""
    per_core = []
    order = np.argsort(dst, kind="stable")
    src, dst, val = src[order], dst[order], val[order]
    core_of = dst // NPC
    core_starts = np.searchsorted(core_of, np.arange(P + 1))
    for c in range(P):
        lo, hi = core_starts[c], core_starts[c + 1]
        s, d, v = src[lo:hi], dst[lo:hi] - c * NPC, val[lo:hi]
        tile_of = d // DT
        tile_starts = np.searchsorted(tile_of, np.arange(NDT + 1))
        groups = []
        for t in range(NDT):
            a, b = tile_starts[t], tile_starts[t + 1]
            st, dl, vt = s[a:b], d[a:b] - t * DT, v[a:b]
            uniq, inv = np.unique(st, return_inverse=True)
            if len(uniq) == 0:
                groups.append((np.zeros(1, np.int64), np.zeros((1, DT), np.float32)))
                continue
            S = np.zeros((len(uniq), DT), np.float32)
            np.add.at(S, (inv, dl), vt)
            groups.append((uniq, S))
        per_core.append(groups)

    ET = [max(max((len(per_core[c][t][0]) + 127) // 128, 1) for c in range(P))
          for t in range(NDT)]
    T = sum(ET)
    off = np.cumsum([0] + ET[:-1])
    idx = np.zeros((P, T, 128), np.int32)
    S_all = np.zeros((P, T, 128, DT), np.float32)
    for c in range(P):
        for t in range(NDT):
            uniq, S = per_core[c][t]
            n = len(uniq)
            o = off[t]
            idx[c, o:o + (n + 127) // 128].reshape(-1)[:n] = _to_padded_id(uniq)
            S_all[c, o:o + (n + 127) // 128].reshape(-1, DT)[:n] = S
    return tuple(ET), idx, S_all


def _idx_dev(idx_core):
    """[T, 128] int32 -> [128, T*8] int16 (wrap 16 partitions, replicate x8)."""
    flat = idx_core.reshape(-1)
    n = len(flat)
    a = np.zeros((16, n // 16), np.int16)
    a[np.arange(n) % 16, np.arange(n) // 16] = flat.astype(np.int16)
    return np.tile(a, (8, 1))


def _s_dev(S_core):
    """[T, 128, DT] -> [128, T*DT] bf16."""
    T = S_core.shape[0]
    return np.ascontiguousarray(
        S_core.transpose(1, 0, 2).reshape(128, T * DT)).astype(BF16)


def _w_dev(W):
    """[F, fo] -> [128, KC*fo] bf16 (chunk k at cols [k*fo, (k+1)*fo))."""
    fi, fo = W.shape
    k = fi // 128
    return np.ascontiguousarray(
        W.reshape(k, 128, fo).transpose(1, 0, 2).reshape(128, k * fo)).astype(BF16)


def _prep(x, edge_index, edge_weight, W1, b1, W2, b2, Wc, bc):
    row = np.asarray(edge_index[0], np.int64)
    col = np.asarray(edge_index[1], np.int64)
    w = np.asarray(edge_weight, np.float32)

    # GCN norm (layers 1 & 2): deg over dest (col) + 1 self loop.
    deg = np.zeros(N, np.float32)
    np.add.at(deg, col, w)
    deg += 1.0
    dinv = (1.0 / np.sqrt(deg)).astype(np.float32)
    g_src = np.concatenate([row, np.arange(N)])
    g_dst = np.concatenate([col, np.arange(N)])
    g_val = np.concatenate([dinv[row] * w * dinv[col], dinv * dinv]).astype(np.float32)

    # Cheb: drop self loops, deg over src (row), negate (lhat = -A_norm).
    keep = row != col
    r0, c0, w0 = row[keep], col[keep], w[keep]
    deg2 = np.zeros(N, np.float32)
    np.add.at(deg2, r0, w0)
    dinv2 = np.where(deg2 > 0, 1.0 / np.sqrt(deg2), 0.0).astype(np.float32)
    c_val = -(dinv2[r0] * w0 * dinv2[c0]).astype(np.float32)

    ETg, idxg, Sg = _build_edge_tiles(g_src, g_dst, g_val)
    ETc, idxc, Sc = _build_edge_tiles(r0, c0, c_val)

    x = np.asarray(x, np.float32)
    x_pad = np.zeros((NTOT, F), BF16)
    x_pad[_to_padded_id(np.arange(N))] = x.astype(BF16)

    Wc = np.asarray(Wc, np.float32)
    com = dict(
        x_bf=x_pad,
        w1=_w_dev(np.asarray(W1, np.float32)),
        w2=_w_dev(np.asarray(W2, np.float32)),
        wa=_w_dev(Wc[0] - Wc[2]),
        wpq=_w_dev(np.concatenate([Wc[1], 2.0 * Wc[2]], axis=1)),
        ident=np.eye(128, dtype=BF16),
    )
    biases = (np.asarray(b1, np.float32), np.asarray(b2, np.float32),
              np.asarray(bc, np.float32))
    in_maps = []
    for c in range(P):
        m = dict(com)
        m["idxg"] = _idx_dev(idxg[c])
        m["sg"] = _s_dev(Sg[c])
        m["idxc"] = _idx_dev(idxc[c])
        m["sc"] = _s_dev(Sc[c])
        in_maps.append(m)
    return ETg, ETc, biases, in_maps


# ------------------------------------------------------------- bass program

_CACHE = {}


def _build_program(ETg, ETc, has_bias):
    import os
    key = (ETg, ETc, has_bias, os.environ.get("GNN_PHASES", "9"))
    if key in _CACHE:
        return _CACHE[key]
    TG, TC = sum(ETg), sum(ETc)
    ETMAX = max(max(ETg), max(ETc))

    nc = bacc.Bacc("TRN2", target_bir_lowering=False, num_devices=P,
                   num_swdge_queues=4)
    x_bf = nc.dram_tensor("x_bf", [NTOT, F], BF16D, kind="ExternalInput")
    idxg = nc.dram_tensor("idxg", [128, TG * 8], I16, kind="ExternalInput")
    sg = nc.dram_tensor("sg", [128, TG * DT], BF16D, kind="ExternalInput")
    idxc = nc.dram_tensor("idxc", [128, TC * 8], I16, kind="ExternalInput")
    sc = nc.dram_tensor("sc", [128, TC * DT], BF16D, kind="ExternalInput")
    w1 = nc.dram_tensor("w1", [128, KC * F], BF16D, kind="ExternalInput")
    w2 = nc.dram_tensor("w2", [128, KC * F], BF16D, kind="ExternalInput")
    wa = nc.dram_tensor("wa", [128, KC * DOUT], BF16D, kind="ExternalInput")
    wpq = nc.dram_tensor("wpq", [128, KC * F], BF16D, kind="ExternalInput")
    ident = nc.dram_tensor("ident", [128, 128], BF16D, kind="ExternalInput")
    if has_bias:
        brows = nc.dram_tensor("brows", [1, 2 * F + DOUT], FP32, kind="ExternalInput")
    outp = nc.dram_tensor("out", [NPAD, DOUT], FP32, kind="ExternalOutput")

    h1c = nc.dram_tensor("h1c", [NPAD, F], BF16D, kind="Internal")
    h1f = nc.dram_tensor("h1f", [NTOT, F], BF16D, kind="Internal", addr_space="Shared")
    y12c = nc.dram_tensor("y12c", [NPAD, F], BF16D, kind="Internal")
    y12f = nc.dram_tensor("y12f", [NTOT, F], BF16D, kind="Internal", addr_space="Shared")
    a2c = nc.dram_tensor("a2c", [NPAD, DOUT], BF16D, kind="Internal")
    a2f = nc.dram_tensor("a2f", [NTOT, DOUT], BF16D, kind="Internal", addr_space="Shared")

    Exp = mybir.ActivationFunctionType.Exp
    Alu = mybir.AluOpType

    with tile.TileContext(nc) as tc:
        with (
            tc.tile_pool(name="const", bufs=1) as cpool,
            tc.tile_pool(name="keep", bufs=1) as kpool,
            tc.tile_pool(name="msgs", bufs=2) as mpool,
            tc.tile_pool(name="msgsn", bufs=3) as npool,
            tc.tile_pool(name="work", bufs=3) as wpool,
            tc.tile_pool(name="psum", bufs=2, space="PSUM") as ppool,
            tc.tile_pool(name="psum3", bufs=3, space="PSUM") as ppool3,
        ):
            lib = nc.gpsimd.load_library(library_config.mlp)
            prep_sems = [nc.alloc_semaphore(f"prep_dma{q}") for q in range(4)]
            sem_clrs = [nc.gpsimd.sem_clear(s) for s in prep_sems]
            first_prep = [0]

            # Load order is priority order on the HWDGE rings: everything
            # layer-1 tile 0 needs first, then the rest.
            ig_sb = cpool.tile([128, TG * 8], I16, tag="ig")
            nc.sync.dma_start(ig_sb[:], idxg[:])
            id_sb = cpool.tile([128, 128], BF16D, tag="id")
            nc.sync.dma_start(id_sb[:], ident[:])

            sg_sb = cpool.tile([128, TG * DT], BF16D, tag="sg")
            sc_sb = cpool.tile([128, TC * DT], BF16D, tag="sc")
            offg = np.cumsum([0] + list(ETg[:-1]))
            offc = np.cumsum([0] + list(ETc[:-1]))
            nc.sync.dma_start(sg_sb[:, :ETg[0] * DT], sg[:, :ETg[0] * DT])
            w1_sb = cpool.tile([128, KC * F], BF16D, tag="w1")
            nc.sync.dma_start(w1_sb[:], w1[:])
            ic_sb = cpool.tile([128, TC * 8], I16, tag="ic")
            nc.sync.dma_start(ic_sb[:], idxc[:])
            for t in range(NDT):
                a, b = offg[t] * DT, (offg[t] + ETg[t]) * DT
                if t > 0:
                    nc.sync.dma_start(sg_sb[:, a:b], sg[:, a:b])
                a, b = offc[t] * DT, (offc[t] + ETc[t]) * DT
                nc.sync.dma_start(sc_sb[:, a:b], sc[:, a:b])
            w2_sb = cpool.tile([128, KC * F], BF16D, tag="w2")
            nc.sync.dma_start(w2_sb[:], w2[:])
            wa_sb = cpool.tile([128, KC * DOUT], BF16D, tag="wa")
            nc.sync.dma_start(wa_sb[:], wa[:])
            wpq_sb = cpool.tile([128, KC * F], BF16D, tag="wpq")
            nc.sync.dma_start(wpq_sb[:], wpq[:])
            if has_bias:
                br_sb = cpool.tile([1, 2 * F + DOUT], FP32, tag="br")
                nc.sync.dma_start(br_sb[:], brows[:])
                ones_sb = cpool.tile([1, 128], FP32, tag="ones")
                nc.vector.memset(ones_sb[:], 1.0)

            h2keep = kpool.tile([128, NDT, F], BF16D, tag="h2k")
            a12keep = kpool.tile([128, NDT, F], BF16D, tag="a12k")

            first_gather = [0]
            qctr = [0]

            def gather_msgs(src_dram, ET, off, idx_sb, t, width, prep=False):
                """Gather the deduped source rows for dest-tile t (width cols).
                Split across SWDGE queues so descriptor generation runs on
                multiple Q7 core pairs in parallel. With prep=True the calls
                are prepare_only: descriptor generation runs as soon as the
                msgs slot frees (Tile defers the src_dram read dep to the
                trigger), so descgen overlaps the preceding allgather."""
                o = off[t]
                et = ET[t]
                if width == F:
                    msgs = mpool.tile([128, ETMAX, width], BF16D, tag="msgs")
                else:
                    msgs = npool.tile([128, ETMAX, width], BF16D, tag="msgsn")
                nq = min(8, et)
                bounds = [et * i // nq for i in range(nq + 1)]
                used = []
                for a, b in zip(bounds[:-1], bounds[1:]):
                    if b <= a:
                        continue
                    q = qctr[0] % 4
                    qctr[0] += 1
                    used.append(q)
                    if prep:
                        prep_pending[q] += 1
                    gi = nc.gpsimd.dma_gather(
                        msgs[:, a:b, :], src_dram[:],
                        idx_sb[:, (o + a) * 8:(o + b) * 8],
                        (b - a) * 128, (b - a) * 128, width,
                        single_packet=True, queue_num=q, prepare_only=prep,
                        sem=prep_sems[q] if prep else None)
                    if prep and first_prep[0] < 8:
                        add_dep_helper(gi.ins, sem_clrs[q].ins,
                                       reason="clear prep sem before preps")
                        first_prep[0] += 1
                    if first_gather[0] < 4:
                        add_dep_helper(gi.ins, lib.ins,
                                       reason="mlp lib before gather")
                        first_gather[0] += 1
                return msgs, sorted(set(used))

            prep_pending = [0, 0, 0, 0]
            prep_fired = [0, 0, 0, 0]

            def fire(queues):
                for q in sorted(set(queues)):
                    nc.gpsimd.trigger_dma(count=None, queue_num=q)
                    prep_fired[q] += prep_pending[q]
                    prep_pending[q] = 0

            def prefetch(src_dram, ET, off, idx_sb, width, n):
                """prepare_only-gather the first n tiles of a pass: descgen
                happens inside the allgather window, the DMAs launch the
                moment the allgather lands. Consumer sync is user-managed
                for preps: returns (msgs, [(q, sem_val)]) guard pairs the
                consuming matmuls must wait_ge on."""
                pre = []
                for t in range(n):
                    m, u = gather_msgs(src_dram, ET, off, idx_sb, t, width,
                                       prep=True)
                    fire(u)
                    guards = [(q, 16 * prep_fired[q]) for q in u]
                    pre.append((m, guards))
                return pre

            def scatter_mms(ps, msgs, ET, off, s_sb, t, start, stop,
                            guards=()):
                # S tile as stationary lhsT, msgs streamed as rhs (N=width):
                # ps[d, f] += S[e, d].T @ msgs[e, f]  (node-major aggregate).
                # guards: prep-DMA (q, sem_val) pairs the first matmul must
                # wait on (prep data-landed sync is user-managed).
                o = off[t]
                et = ET[t]
                waits = [nc.tensor.wait_ge(prep_sems[q], v)
                         for q, v in guards]
                for g in range(et):
                    mm = nc.tensor.matmul(
                        ps[:],
                        s_sb[:, (o + g) * DT:(o + g + 1) * DT],
                        msgs[:, g, :],
                        start=(start and g == 0), stop=(stop and g == et - 1))
                    if g == 0:
                        for w in waits:
                            add_dep_helper(mm.ins, w.ins,
                                           reason="prep gather landed")

            def scatter(src_dram, ET, off, idx_sb, s_sb, t, width=F,
                        pre=None):
                guards = ()
                if pre is not None and t < len(pre):
                    msgs, guards = pre[t]
                else:
                    msgs, _ = gather_msgs(src_dram, ET, off, idx_sb, t, width)
                ps = ppool3.tile([128, width], FP32, tag="psT")
                scatter_mms(ps, msgs, ET, off, s_sb, t, True, True,
                            guards=guards)
                return ps

            def celu(z_ps, width, out_ap):
                """out = max(z,0) + min(exp(z)-1, 0); z read from PSUM.
                fp32 temporaries: bf16 here doubles the output error
                (exp(z)-1 cancellation) for no measured speed gain."""
                e = wpool.tile([128, F], FP32, tag="e")
                nc.scalar.activation(e[:, :width], z_ps, Exp)
                em = wpool.tile([128, F], FP32, tag="em")
                nc.vector.tensor_scalar(
                    em[:, :width], e[:, :width], 1.0, 0.0,
                    Alu.subtract, Alu.min)
                nc.vector.scalar_tensor_tensor(
                    out_ap, z_ps, 0.0, em[:, :width], Alu.max, Alu.add)

            def gemm_bias(z_ps, width, b_off):
                if has_bias:
                    nc.tensor.matmul(
                        z_ps, ones_sb[:],
                        br_sb[:, b_off:b_off + width],
                        start=False, stop=False)

            def allgather_chunk(cin, cout, j):
                nc.gpsimd.collective_compute(
                    "AllGather", Alu.bypass,
                    replica_groups=[list(range(P))],
                    ins=[cin[j * CH:(j + 1) * CH, :]],
                    outs=[cout[j * P * CH:(j + 1) * P * CH, :]])

            def gcn_layer(src_dram, w_sb, dst_dram, keep_tile, b_off, post,
                          pre=None):
                for t in range(NDT):
                    ps = scatter(src_dram, ETg, offg, ig_sb, sg_sb, t, pre=pre)
                    agg = wpool.tile([128, F], BF16D, tag="agg")
                    nc.vector.tensor_copy(agg[:], ps[:])
                    tps = ppool.tile([128, KC, 128], BF16D, tag="tps")
                    for k in range(KC):
                        nc.tensor.transpose(
                            tps[:, k, :], agg[:, k * 128:(k + 1) * 128], id_sb[:])
                    aggT = wpool.tile([128, KC, 128], BF16D, tag="aggT")
                    nc.vector.tensor_copy(aggT[:], tps[:])
                    z = ppool.tile([128, F], FP32, tag="z")
                    for k in range(KC):
                        nc.tensor.matmul(
                            z[:], aggT[:, k, :], w_sb[:, k * F:(k + 1) * F],
                            start=(k == 0), stop=(k == KC - 1))
                    gemm_bias(z[:], F, b_off)
                    if keep_tile is None:
                        h = wpool.tile([128, F], BF16D, tag="h")
                        celu(z[:], F, h[:])
                        nc.sync.dma_start(dst_dram[t * 128:(t + 1) * 128, :], h[:])
                    else:
                        celu(z[:], F, keep_tile[:, t, :])
                        if dst_dram is not None:
                            nc.sync.dma_start(dst_dram[t * 128:(t + 1) * 128, :],
                                              keep_tile[:, t, :])
                    if post is not None:
                        post(t)

            import os
            PH = int(os.environ.get("GNN_PHASES", "9"))

            def post1(t):
                if PH >= 2 and (t * 128 + 128) % CH == 0:
                    allgather_chunk(h1c, h1f, (t * 128 + 128) // CH - 1)

            def post2(t):
                # Transpose h2 tile in place to feature-major, then the
                # cheb pre-GEMM: y12 = h2 @ [Wk1 | 2*Wk2]  (node-major out).
                tps = ppool.tile([128, KC, 128], BF16D, tag="tps")
                for k in range(KC):
                    nc.tensor.transpose(
                        tps[:, k, :], h2keep[:, t, k * 128:(k + 1) * 128],
                        id_sb[:])
                nc.vector.tensor_copy(h2keep[:, t, :], tps[:])
                zy = ppool.tile([128, F], FP32, tag="z")
                for k in range(KC):
                    nc.tensor.matmul(
                        zy[:], h2keep[:, t, k * 128:(k + 1) * 128],
                        wpq_sb[:, k * F:(k + 1) * F],
                        start=(k == 0), stop=(k == KC - 1))
                ynm = wpool.tile([128, F], BF16D, tag="h")
                nc.vector.tensor_copy(ynm[:], zy[:])
                nc.sync.dma_start(y12c[t * 128:(t + 1) * 128, :], ynm[:])
                if PH >= 4 and (t * 128 + 128) % CH == 0:
                    allgather_chunk(y12c, y12f, (t * 128 + 128) // CH - 1)

            # ---- layer 1: h1 = celu((Ag @ x) @ W1 + b1)
            # (prepare_only descgen prefetch measured net-negative: the
            # trigger waits block the SWDGE queue FIFOs. PRE_N=0 disables.)
            PRE_N = 0
            pre1 = prefetch(x_bf, ETg, offg, ig_sb, F, PRE_N) or None
            gcn_layer(x_bf, w1_sb, h1c, None, 0, post1, pre=pre1)

            # ---- layer 2: h2 = celu((Ag @ h1) @ W2 + b2); keep h2 on chip
            # (feature-major after post2) and emit y12 = h2 @ [Wk1 | 2*Wk2].
            if PH >= 3:
                pre2 = prefetch(h1f, ETg, offg, ig_sb, F, PRE_N) or None
                gcn_layer(h1f, w2_sb, None, h2keep, F, post2, pre=pre2)

            # ---- cheb pass B: [A1|A2] = lhat([Y1|Y2]); keep on chip,
            # allgather A2 (256 wide) for pass C.
            if PH >= 5:
                preb = prefetch(y12f, ETc, offc, ic_sb, F, PRE_N) or None
                for t in range(NDT):
                    ps = scatter(y12f, ETc, offc, ic_sb, sc_sb, t, pre=preb)
                    nc.vector.tensor_copy(a12keep[:, t, :], ps[:])
                    nc.sync.dma_start(a2c[t * 128:(t + 1) * 128, :],
                                      a12keep[:, t, DOUT:])
                    if PH >= 6 and (t * 128 + 128) % CH == 0:
                        allgather_chunk(a2c, a2f, (t * 128 + 128) // CH - 1)

            # ---- cheb pass C + output:
            # out = celu(h2 @ (Wk0-Wk2) + A1 + lhat(A2) + bc)
            prec = prefetch(a2f, ETc, offc, ic_sb, DOUT, PRE_N) if PH >= 7 else []
            for t in range(NDT if PH >= 7 else 0):
                zo = ppool.tile([128, DOUT], FP32, tag="z")
                for k in range(KC):
                    # h2keep is feature-major (transposed in post2)
                    nc.tensor.matmul(
                        zo[:], h2keep[:, t, k * 128:(k + 1) * 128],
                        wa_sb[:, k * DOUT:(k + 1) * DOUT],
                        start=(k == 0), stop=False)
                gemm_bias(zo[:], DOUT, 2 * F)
                # zo += A1 via identity-stationary matmul (PE add, no DVE)
                nc.tensor.matmul(
                    zo[:], id_sb[:], a12keep[:, t, :DOUT],
                    start=False, stop=False)
                guards = ()
                if t < len(prec):
                    msgs, guards = prec[t]
                else:
                    msgs, _ = gather_msgs(a2f, ETc, offc, ic_sb, t, DOUT)
                scatter_mms(zo, msgs, ETc, offc, sc_sb, t, False, True,
                            guards=guards)
                of = wpool.tile([128, DOUT], FP32, tag="of")
                celu(zo[:], DOUT, of[:])
                nc.sync.dma_start(outp[t * 128:(t + 1) * 128, :], of[:])

    nc.compile()
    _CACHE[key] = nc
    return nc


# ------------------------------------------------------------------- driver

def _run(inputs, trace=False, tmpdir=None):
    ETg, ETc, biases, in_maps = _prep(**inputs)
    has_bias = any(np.any(b != 0) for b in biases)
    if has_bias:
        brow = np.concatenate(biases).astype(np.float32)[None, :]
        for m in in_maps:
            m["brows"] = brow
    nc = _build_program(ETg, ETc, has_bias)
    res = run_bass_kernel_spmd(nc, in_maps, core_ids=list(range(P)),
                               trace=trace, tmpdir=tmpdir)
    out = np.concatenate(
        [res.results[c]["out"][:NPC] for c in range(P)], axis=0)
    return out.astype(np.float32), res


def kernel(**inputs) -> np.ndarray:
    out, _ = _run(inputs)
    return out

